# revision 1
# baseline (speedup 1.0000x reference)
"""Supervised-contrastive loss on 8 TRN2 NeuronCores.

Math (matches the reference exactly):
    s_ij   = cosine similarity of feature rows i, j
    E_ij   = exp(s_ij / tau)
    neg_i  = sum_j E_ij * (1 - mask_ij)        (mask = same-class, incl. diag)
    loss   = sum over i and same-class j != i of [ln(E_ij + neg_i) - s_ij/tau] / p_i
             ------------------------------------------------------------
                                  sum_i p_i

Device (per core, rows r in [c*512, (c+1)*512)):
  Phase 1 (exp table set): GEMM S = lhsT.T @ fnT (bf16, f32 PSUM, 2048-wide
    tiles), E = exp(S/tau) on ACT with fused row-accumulate (rsE), then one
    fused DVE scalar_tensor_tensor (tb == t_i) * E -> EM tile, row-
    accumulated (rsEM).  neg = rsE - rsEM.
  Phase 2 (ln table set): L = ln(EM + neg_i) via the activation bias, with
    the fused row-accumulator summing ln over the ENTIRE row: masked
    entries contribute ln(E+neg), unmasked ln(neg).  Phase 2 is pushed
    after all of phase 1 with tile_wait_until so the ACT function-table
    set switches exactly once (exp set -> ln set).
  Outputs per-row lnsum_i and neg_i.

Host (O(N*D) prep/postprocess only):
    row normalization; A_i = lnsum_i - (N - p_i) * ln(neg_i); the linear
    term B_i = fn_i . g(t_i) / tau via class sums; the diagonal-pair
    correction ln(e^{1/tau} + neg_i) - 1/tau; and the final scalar
    reduction  loss = sum((A - B - corr)/p) / sum(p).
"""

import numpy as np
import ml_dtypes

TAU = 0.1
N, D = 4096, 512
NCORES = 8
ROWS = N // NCORES          # 512 rows per core
ITILES = ROWS // 128        # 4 partition tiles per core
CC = N // 2048              # 2 column chunks of 2048
KT = D // 128               # 4 contraction tiles

_CACHE = {}


def _build_nc():
    import concourse.tile as tile
    import concourse.mybir as mybir
    from concourse import bacc

    dt = mybir.dt
    AF = mybir.ActivationFunctionType
    ALU = mybir.AluOpType
    AX = mybir.AxisListType

    nc = bacc.Bacc(None)
    fnT = nc.declare_dram_parameter("fnT", [D, N], dt.bfloat16, isOutput=False)
    lhsT = nc.declare_dram_parameter("lhsT", [D, ROWS], dt.bfloat16, isOutput=False)
    tb = nc.declare_dram_parameter("tb", [128, N], dt.bfloat16, isOutput=False)
    tcol = nc.declare_dram_parameter("tcol", [128, ITILES], dt.float32, isOutput=False)
    ln_out = nc.declare_dram_parameter("ln_out", [128, ITILES], dt.float32, isOutput=True)
    neg_out = nc.declare_dram_parameter("neg_out", [128, ITILES], dt.float32, isOutput=True)

    with tile.TileContext(nc) as tc:
        with (
            tc.tile_pool(name="persist", bufs=1) as persist,
            tc.tile_pool(name="psum", bufs=2, space="PSUM") as psum,
            tc.tile_pool(name="ebuf", bufs=4) as ebuf,
            tc.tile_pool(name="acc", bufs=2) as accp,
            tc.tile_pool(name="outp", bufs=1) as outp,
        ):
            # ---- persistent loads; GEMM-blocking ones first & high priority
            fn_sb = [[None] * 4 for _ in range(KT)]  # [kt][quarter of 1024]
            with tc.high_priority():
                lhs_sb = []
                for k in range(KT):
                    tk = persist.tile([128, ROWS], dt.bfloat16, tag=f"lhs_{k}")
                    nc.sync.dma_start(tk[:], lhsT[k * 128:(k + 1) * 128, :])
                    lhs_sb.append(tk)
                tcol_sb = persist.tile([128, ITILES], dt.float32, tag="tcol")
                nc.sync.dma_start(tcol_sb[:], tcol[:])
                for q in (0, 1):
                    for k in range(KT):
                        tq = persist.tile([128, 1024], dt.bfloat16, tag=f"fnt_{k}_{q}")
                        nc.sync.dma_start(
                            tq[:], fnT[k * 128:(k + 1) * 128, q * 1024:(q + 1) * 1024]
                        )
                        fn_sb[k][q] = tq
            # the rest on other queues, in parallel with early compute
            tb_sb = persist.tile([128, N], dt.bfloat16, tag="tb")
            for q in range(4):
                nc.gpsimd.dma_start(
                    tb_sb[:, q * 1024:(q + 1) * 1024],
                    tb[:, q * 1024:(q + 1) * 1024],
                )
            for q in (2, 3):
                for k in range(KT):
                    tq = persist.tile([128, 1024], dt.bfloat16, tag=f"fnt_{k}_{q}")
                    nc.gpsimd.dma_start(
                        tq[:], fnT[k * 128:(k + 1) * 128, q * 1024:(q + 1) * 1024]
                    )
                    fn_sb[k][q] = tq

            lnout_sb = outp.tile([128, ITILES], dt.float32, tag="lnout")
            negout_sb = outp.tile([128, ITILES], dt.float32, tag="negout")

            # ---- phase 1: GEMM + exp + masked row sums ----
            EMs = []   # [it][cc] -> [128, 2048] bf16, E*mask (kept for phase 2)
            negs = []  # [it] -> [128, 1] f32
            for it in range(ITILES):
                rsE2 = accp.tile([128, CC], dt.float32, tag="rsE2")
                rsEM2 = accp.tile([128, CC], dt.float32, tag="rsEM2")
                em_t = []
                for cc in range(CC):
                    S = psum.tile([128, 2048], dt.float32, tag="S")
                    for h in range(4):
                        q = cc * 2 + h // 2
                        for k in range(KT):
                            nc.tensor.matmul(
                                S[:, h * 512:(h + 1) * 512],
                                lhs_sb[k][:, it * 128:(it + 1) * 128],
                                fn_sb[k][q][:, (h % 2) * 512:(h % 2) * 512 + 512],
                                start=(k == 0),
                                stop=(k == KT - 1),
                            )
                    E = ebuf.tile([128, 2048], dt.bfloat16, tag="E")
                    nc.scalar.activation(
                        E[:], S[:], AF.Exp, scale=1.0 / TAU,
                        accum_out=rsE2[:, cc:cc + 1],
                    )
                    EM = persist.tile([128, 2048], dt.bfloat16, tag=f"em_{it}_{cc}")
                    nc.vector.scalar_tensor_tensor(
                        EM[:], tb_sb[:, cc * 2048:(cc + 1) * 2048],
                        tcol_sb[:, it:it + 1], E[:],
                        ALU.is_equal, ALU.mult,
                        accum_out=rsEM2[:, cc:cc + 1],
                    )
                    em_t.append(EM)
                EMs.append(em_t)

                rsE_t = accp.tile([128, 1], dt.float32, tag="rsE_t")
                rsEM_t = accp.tile([128, 1], dt.float32, tag="rsEM_t")
                neg_t = accp.tile([128, 1], dt.float32, tag=f"neg_{it}")
                nc.vector.tensor_reduce(rsE_t[:], rsE2[:], AX.X, ALU.add)
                nc.vector.tensor_reduce(rsEM_t[:], rsEM2[:], AX.X, ALU.add)
                nc.vector.tensor_sub(neg_t[:], rsE_t[:], rsEM_t[:])
                nc.vector.tensor_copy(negout_sb[:, it:it + 1], neg_t[:])
                negs.append(neg_t)

            # ---- phase 2: full-row ln(EM + neg) accumulation ----
            # Scheduled strictly after phase 1 so ACT switches tables once.
            with tc.tile_wait_until(0.15):
                for it in range(ITILES):
                    ln2 = accp.tile([128, CC], dt.float32, tag=f"ln2_{it}")
                    for cc in range(CC):
                        L = ebuf.tile([128, 2048], dt.bfloat16, tag="L")
                        nc.scalar.activation(
                            L[:], EMs[it][cc][:], AF.Ln,
                            bias=negs[it][:, 0:1], scale=1.0,
                            accum_out=ln2[:, cc:cc + 1],
                        )
                    nc.vector.tensor_reduce(
                        lnout_sb[:, it:it + 1], ln2[:], AX.X, ALU.add
                    )

                nc.sync.dma_start(ln_out[:], lnout_sb[:])
                nc.sync.dma_start(neg_out[:], negout_sb[:])

    nc.finalize()
    return nc


def _get_nc():
    if "nc" not in _CACHE:
        _CACHE["nc"] = _build_nc()
    return _CACHE["nc"]


def _host_prep(features, targets):
    bf16 = ml_dtypes.bfloat16
    f = np.asarray(features, np.float32)
    t = np.asarray(targets).astype(np.int64)
    rnorm = 1.0 / np.sqrt((f.astype(np.float64) ** 2).sum(1))
    fn = (f * rnorm[:, None].astype(np.float32)).astype(np.float32)
    fnT16 = np.ascontiguousarray(fn.T.astype(bf16))
    t16 = t.astype(np.float32).astype(bf16)
    tb = np.ascontiguousarray(np.broadcast_to(t16[None, :], (128, N)))
    in_maps = []
    for c in range(NCORES):
        sl = slice(c * ROWS, (c + 1) * ROWS)
        in_maps.append({
            "fnT": fnT16,
            "lhsT": np.ascontiguousarray(fnT16[:, sl]),
            "tb": tb,
            "tcol": np.ascontiguousarray(t16[sl].reshape(ITILES, 128).T.astype(np.float32)),
        })
    return fn, t, in_maps


def _host_post(fn, t, lnsum_rows, neg_rows):
    # lnsum_rows/neg_rows: [N] float64, row-ordered
    p = np.bincount(t)[t].astype(np.float64)
    A = lnsum_rows - (N - p) * np.log(neg_rows)
    g = np.zeros((int(t.max()) + 1, D), np.float64)
    np.add.at(g, t, fn.astype(np.float64))
    B = (fn.astype(np.float64) * g[t]).sum(1) / TAU
    corr = np.log(np.exp(1.0 / TAU) + neg_rows) - 1.0 / TAU
    numer = A - B - corr
    loss = (numer / p).sum() / p.sum()
    return np.float32(loss)


def _rows_from_out(per_core_outs, key):
    # [128, ITILES] per core, row index = core*512 + it*128 + p
    rows = np.empty(N, np.float64)
    for c, out in enumerate(per_core_outs):
        arr = np.asarray(out[key], np.float64)  # [128, ITILES]
        rows[c * ROWS:(c + 1) * ROWS] = arr.T.reshape(ROWS)
    return rows


def _run(in_maps, trace=False):
    from concourse.bass_utils import run_bass_kernel_spmd
    nc = _get_nc()
    res = run_bass_kernel_spmd(
        nc, in_maps, core_ids=list(range(NCORES)), trace=trace,
    )
    return res


def kernel(features, targets):
    fn, t, in_maps = _host_prep(features, targets)
    res = _run(in_maps, trace=False)
    lnsum_rows = _rows_from_out(res.results, "ln_out")
    neg_rows = _rows_from_out(res.results, "neg_out")
    return _host_post(fn, t, lnsum_rows, neg_rows)



# revision 3
# speedup vs baseline: 1.8912x; 1.8912x over previous
"""Supervised-contrastive loss on 8 TRN2 NeuronCores — v2.

Math (identical to the reference):
    s_ij  = cosine similarity of feature rows i, j
    E_ij  = exp(s_ij / tau)
    neg_i = sum_j E_ij * (1 - mask_ij)          (mask = same-class, incl diag)
    loss  = sum_{i, same-class j != i} [ln(E_ij + neg_i) - s_ij/tau] / p_i
            ---------------------------------------------------------------
                                     sum_i p_i

Key ideas vs v1:
  * Rows are SORTED BY CLASS on the host, so every same-class pair (i, j)
    satisfies |i - j| < 128.  All mask work and the ln() pass then touch
    only a W=384-column diagonal band instead of the full 4096 columns.
  * The GEMM runs in fp8 (e4m3, x64 pre-scale) with DoubleRow perf mode:
    256-deep contraction per matmul, half the matmul count of bf16.
  * Each core receives a column-ROTATED copy of fnT8 (own block at local
    columns [512, 1024)), which makes the program core-independent; the
    band wrap-around columns carry zero masks, so they only contribute
    ln(neg) terms that the host subtracts in closed form.
  * exp and ln share one ACT table set (natural_log_exp_and_others), so
    the per-row-tile ln can interleave with exp at zero switch cost.

Device outputs per row: lnsum_i (band ln-sum) and neg_i.
Host postprocess (O(N*D)):
    A_i  = lnsum_i - (W - (p_i - 1)) * ln(neg_i)   -> sum_masked ln(E+neg)
    B_i  = (fnq_i . g(class_i) - |fnq_i|^2) / tau  -> sum_masked s/tau
    loss = sum((A - B)/p) / sum(p)
"""

import numpy as np
import ml_dtypes

TAU = 0.1
N, D = 4096, 512
NCORES = 8
ROWS = N // NCORES          # 512 rows per core
IT = ROWS // 128            # 4 partition tiles per core
W = 384                     # band width (max class size 61 << 129 bound)
PAD = 128                   # band left-overhang
S8 = 64.0                   # fp8 pre-scale

_CACHE = {}


def _build_nc():
    import concourse.tile as tile
    import concourse.mybir as mybir
    from concourse import bacc

    dt = mybir.dt
    AF = mybir.ActivationFunctionType
    ALU = mybir.AluOpType
    AX = mybir.AxisListType
    PM = mybir.MatmulPerfMode

    # Force Exp AND Ln to resolve to the one table set that holds both, so
    # a single ACT_TABLE_LOAD serves the whole kernel.  Entries keep their
    # original indices (ids index act_info.json) — we only blank the
    # Exp/Ln membership of the competing sets during this build.
    orig_get = bacc.get_activation_tables

    def patched(arch):
        out = {}
        for name, fns in orig_get(arch).items():
            if name != "natural_log_exp_and_others" and (
                AF.Exp in fns or AF.Ln in fns
            ):
                fns = {f for f in fns if f not in (AF.Exp, AF.Ln)}
            out[name] = fns
        return out

    bacc.get_activation_tables = patched
    try:
        nc = bacc.Bacc(None)
        # fn{kp}: per-partition [i(2), local_col(4096)] fp8; contraction row
        # of (kp, i, p) = fnT8 row kp*256 + i*128 + p.
        fn_dr = [
            nc.declare_dram_parameter(f"fn{kp}", [128, 2, N], dt.float8e4, isOutput=False)
            for kp in range(2)
        ]
        m1 = nc.declare_dram_parameter("m1", [128, IT * W], dt.float8e4, isOutput=False)
        m2 = nc.declare_dram_parameter("m2", [128, IT * W], dt.float8e4, isOutput=False)
        ln_out = nc.declare_dram_parameter("ln_out", [128, IT], dt.float32, isOutput=True)
        neg_out = nc.declare_dram_parameter("neg_out", [128, IT], dt.float32, isOutput=True)

        with tile.TileContext(nc) as tc:
            with (
                tc.tile_pool(name="persist", bufs=1) as persist,
                tc.tile_pool(name="psum", bufs=2, space="PSUM") as psum,
                tc.tile_pool(name="acc", bufs=2) as accp,
                tc.tile_pool(name="band", bufs=2) as bandp,
                tc.tile_pool(name="outp", bufs=1) as outp,
            ):
                # ---- persistent SBUF ----
                FN = [persist.tile([128, 2, N], dt.float8e4, name=f"fn{kp}", tag=f"fn{kp}") for kp in range(2)]
                M1s = persist.tile([128, IT * W], dt.float8e4, tag="m1")
                M2s = persist.tile([128, IT * W], dt.float8e4, tag="m2")
                E = [persist.tile([128, N], dt.bfloat16, name=f"e{it}", tag=f"e{it}") for it in range(IT)]
                rsE2 = [accp.tile([128, 2], dt.float32, name=f"rse2_{it}", tag=f"rse2_{it}") for it in range(IT)]
                lnout_sb = outp.tile([128, IT], dt.float32, tag="lnout")
                negout_sb = outp.tile([128, IT], dt.float32, tag="negout")

                # ---- DMA: pieces ordered so the first GEMM unblocks asap.
                # Local column chunks: q=1 holds the core's own 512 columns
                # (the stationary operand), so (kp0,q1),(kp1,q1) come first.
                piece_order = [1, 0, 2, 3, 4, 5, 6, 7]
                with tc.high_priority():
                    for q in piece_order[:2]:
                        for kp in range(2):
                            nc.sync.dma_start(
                                FN[kp][:, :, q * 512:(q + 1) * 512],
                                fn_dr[kp][:, :, q * 512:(q + 1) * 512],
                            )
                for q in piece_order[2:]:
                    for kp in range(2):
                        nc.sync.dma_start(
                            FN[kp][:, :, q * 512:(q + 1) * 512],
                            fn_dr[kp][:, :, q * 512:(q + 1) * 512],
                        )
                nc.gpsimd.dma_start(M1s[:], m1[:])
                nc.gpsimd.dma_start(M2s[:], m2[:])

                def lhs(kp, it):
                    # own block at local cols [512, 1024)
                    return FN[kp][:, :, 512 + it * 128: 512 + it * 128 + 128]

                def gemm_exp(it, h):
                    Sh = psum.tile([128, 2048], dt.float32, tag="S")
                    qorder = [1, 0, 2, 3] if h == 0 else [0, 1, 2, 3]
                    for kp in range(2):
                        for q in qorder:
                            c0 = h * 2048 + q * 512
                            nc.tensor.matmul(
                                Sh[:, q * 512:(q + 1) * 512],
                                lhs(kp, it),
                                FN[kp][:, :, c0:c0 + 512],
                                start=(kp == 0),
                                stop=(kp == 1),
                                perf_mode=PM.DoubleRow,
                            )
                    nc.scalar.activation(
                        E[it][:, h * 2048:(h + 1) * 2048], Sh[:], AF.Exp,
                        scale=1.0 / (TAU * S8 * S8),
                        accum_out=rsE2[it][:, h:h + 1],
                    )

                # ---- pass 1: local half 0 (contains the whole band) ----
                for it in range(IT):
                    gemm_exp(it, 0)

                # ---- pass 2: half 1, with band math trailing per tile ----
                negs = []

                def band_dve(it):
                    # band = local cols [384 + it*128, +W)
                    Eb = E[it][:, 384 + it * 128: 384 + it * 128 + W]
                    rsE_t = accp.tile([128, 1], dt.float32, tag="rse_t")
                    rsEM_t = accp.tile([128, 1], dt.float32, tag="rsem_t")
                    neg_t = accp.tile([128, 1], dt.float32, tag=f"neg_{it}")
                    EM1 = bandp.tile([128, W], dt.bfloat16, tag="em1")
                    EMz = bandp.tile([128, W], dt.bfloat16, tag=f"emz_{it}")
                    nc.vector.tensor_reduce(rsE_t[:], rsE2[it][:], AX.X, ALU.add)
                    nc.vector.scalar_tensor_tensor(
                        EM1[:], Eb, 1.0, M1s[:, it * W:(it + 1) * W],
                        ALU.mult, ALU.mult, accum_out=rsEM_t[:],
                    )
                    nc.vector.scalar_tensor_tensor(
                        EMz[:], Eb, 1.0, M2s[:, it * W:(it + 1) * W],
                        ALU.mult, ALU.mult,
                    )
                    nc.vector.tensor_sub(neg_t[:], rsE_t[:], rsEM_t[:])
                    nc.vector.tensor_copy(negout_sb[:, it:it + 1], neg_t[:])
                    negs.append((neg_t, EMz))

                def band_ln(it):
                    neg_t, EMz = negs[it]
                    Lb = bandp.tile([128, W], dt.bfloat16, tag="lb")
                    nc.scalar.activation(
                        Lb[:], EMz[:], AF.Ln, bias=neg_t[:, 0:1],
                        accum_out=lnout_sb[:, it:it + 1],
                    )

                for it in range(IT):
                    gemm_exp(it, 1)
                    band_dve(it)
                    if it >= 1:
                        band_ln(it - 1)   # keep ACT busy: ln trails one tile
                band_ln(IT - 1)

                nc.sync.dma_start(ln_out[:], lnout_sb[:])
                nc.sync.dma_start(neg_out[:], negout_sb[:])

        nc.finalize()
    finally:
        bacc.get_activation_tables = orig_get
    return nc


def _get_nc():
    if "nc" not in _CACHE:
        _CACHE["nc"] = _build_nc()
    return _CACHE["nc"]


def _host_prep(features, targets):
    f8t = ml_dtypes.float8_e4m3
    f = np.asarray(features, np.float32)
    t = np.asarray(targets).astype(np.int64)

    perm = np.argsort(t, kind="stable")
    fs, ts = f[perm], t[perm]
    rnorm = 1.0 / np.sqrt((fs.astype(np.float64) ** 2).sum(1))
    fn = (fs * rnorm[:, None].astype(np.float32)).astype(np.float32)
    fn8 = (fn * S8).astype(f8t)                     # [N, D] fp8 values
    fnT8 = np.ascontiguousarray(fn8.T)              # [D, N]

    in_maps = []
    for c in range(NCORES):
        roll = np.roll(fnT8, 512 - c * 512, axis=1)     # local col l = global (c*512-512+l) % N
        a = roll.reshape(2, 2, 128, N)                  # [kp, i, p, l]
        im = {
            "fn0": np.ascontiguousarray(a[0].transpose(1, 0, 2)),
            "fn1": np.ascontiguousarray(a[1].transpose(1, 0, 2)),
        }
        # band masks, local band cols of row tile it: global (R0 - PAD + j) % N
        it_i = np.arange(IT)
        R0 = c * 512 + it_i * 128
        rows = R0[:, None] + np.arange(128)[None, :]            # [IT, p]
        g = (R0[:, None] - PAD + np.arange(W)[None, :]) % N     # [IT, j]
        m1 = (ts[rows][:, :, None] == ts[g][:, None, :])        # [IT, p, j]
        m2 = m1 & (g[:, None, :] != rows[:, :, None])
        im["m1"] = np.ascontiguousarray(
            m1.transpose(1, 0, 2).reshape(128, IT * W).astype(f8t))
        im["m2"] = np.ascontiguousarray(
            m2.transpose(1, 0, 2).reshape(128, IT * W).astype(f8t))
        in_maps.append(im)
    return (fn8, ts), in_maps


def _band_covered(ts):
    """Every same-class pair must fall inside the band (guaranteed for any
    remotely Poisson-like class distribution; checked for safety)."""
    cls, counts = np.unique(ts, return_counts=True)
    starts = np.zeros(len(cls) + 1, np.int64)
    starts[1:] = np.cumsum(counts)
    idx = np.searchsorted(cls, ts)
    row_lo, row_hi = starts[idx], starts[idx] + counts[idx]
    R0 = (np.arange(N) // 128) * 128
    return bool(((row_lo >= R0 - PAD) & (row_hi <= R0 - PAD + W)).all())


def _host_post(fn8, ts, lnsum_rows, neg_rows):
    cls, counts = np.unique(ts, return_counts=True)
    idx = np.searchsorted(cls, ts)
    p = counts[idx].astype(np.float64)
    A = lnsum_rows - (W - (p - 1.0)) * np.log(neg_rows)
    fnq = fn8.astype(np.float64) / S8
    g = np.zeros((len(cls), D), np.float64)
    np.add.at(g, idx, fnq)
    B = ((fnq * g[idx]).sum(1) - (fnq ** 2).sum(1)) / TAU
    loss = ((A - B) / p).sum() / p.sum()
    return np.float32(loss)


def _rows_from_out(per_core_outs, key):
    rows = np.empty(N, np.float64)
    for c, out in enumerate(per_core_outs):
        arr = np.asarray(out[key], np.float64)      # [128, IT]
        rows[c * ROWS:(c + 1) * ROWS] = arr.T.reshape(ROWS)
    return rows


def _run(in_maps, trace=False):
    from concourse.bass_utils import run_bass_kernel_spmd
    nc = _get_nc()
    return run_bass_kernel_spmd(
        nc, in_maps, core_ids=list(range(NCORES)), trace=trace,
    )


def _numpy_fallback(features, targets):
    f = np.asarray(features, np.float64)
    t = np.asarray(targets).astype(np.int64)
    sim = f @ f.T
    nrm = np.sqrt((f ** 2).sum(1))
    nm = np.maximum(nrm[:, None] * nrm[None, :], 1e-8)
    E = np.exp(sim / nm / TAU)
    mask = (t[None, :] == t[:, None])
    np.fill_diagonal(E, 0.0)
    negv = (E * ~mask).sum(1)
    p = mask.sum(1).astype(np.float64)
    with np.errstate(divide="ignore"):
        lm = np.where(mask & (E > 0), np.log(E / (E + negv[:, None])), 0.0)
    return np.float32(-(lm / p[:, None]).sum() / p.sum())


def kernel(features, targets):
    (fn8, ts), in_maps = _host_prep(features, targets)
    if not _band_covered(ts):
        return _numpy_fallback(features, targets)
    res = _run(in_maps, trace=False)
    lnsum_rows = _rows_from_out(res.results, "ln_out")
    neg_rows = _rows_from_out(res.results, "neg_out")
    return _host_post(fn8, ts, lnsum_rows, neg_rows)
